# revision 42
# baseline (speedup 1.0000x reference)
"""Trainium2 Bass kernel: causal attention block with query-axis softmax.

Reference math (per batch element b):
    Q = X @ Wq + bq ; K = X @ Wk + bk ; V = X @ Wv + bv          # [T, D]
    logits[i, j] = Q[i] . K[j],  logits[i, j] = -inf where j > i
    probs = softmax(logits, axis=i) / sqrt(1024)                 # QUERY axis
    out = X + probs @ V

Distribution: pure data-parallel — B=8 batch elements, one per NeuronCore,
weights replicated, no collectives.

Per-core implementation notes (zero-bias fast path):
  * Works in "transposed logit" space LT[j, i] = logits[i, j], so the
    axis-i softmax is a per-partition free-axis reduction.
  * logits = X (Wq Wk^T) X^T: M = Wq Wk^T is computed once on device,
    then Y^T = M^T X^T and LT = X Y^T — one projection instead of two.
  * ALL matmul phases in fp8 + DoubleRow (2 contraction rows per PE
    pass): the host supplies X^T, 32*Wq^T, 32*Wk^T, 32*Wv in e4m3; M
    (stored as 32*M) and Y^T (stored as Y) are requantized to e4m3 on
    device via scaled Scalar-engine copies, so the logits accumulate at
    true scale and the softmax path is unchanged.  probs/V in e5m2.
  * The causal tri-mask is added by the PE itself (identity x mask
    matmul into the diagonal PSUM window) — no Vector-engine touch of
    the accumulator before the softmax reduction.
  * PSUM: logit accumulators are [128, 1024] parts from a 3-deep pool
    (a C row uses 1-2 parts; per-part max/exp chains release PSUM at
    part granularity), probs@V accumulators live in their own bank pair
    — consecutive C rows never rotate into a buffer whose softmax is
    still draining.
  * Causal structure: LT row-chunk jc computes only i >= 128*jc; probs
    rows live in pair-tiles so the DoubleRow probs@V matmuls skip
    fully-masked pairs.  C rows and E columns are interleaved lag-3
    (E_k traced after C_{k+3}) so the PE always has ready matmuls while
    softmax chains drain; a few throwaway matmuls at trace start keep
    the PE activity monitor from re-throttling during the initial
    weight DMA.
  * V = 32*X@Wv is cast to e5m2 by the Scalar engine (Vector stays on
    the softmax chain); the softmax denominator, the 1/sqrt(1024) scale
    and the 1/32 weight prescale fold into V's rows per C phase.

The with_bias=True fallback (never taken for this problem's all-zero
biases) keeps the direct Q/K projection structure with PE transposes.
"""

import sys

if "/opt/trn_rl_repo" not in sys.path:
    sys.path.insert(0, "/opt/trn_rl_repo")

import numpy as np

import concourse.bass as bass
import concourse.mybir as mybir
import concourse.tile as tile
from concourse import bacc
from concourse.bass import ts
from concourse.bass_utils import run_bass_kernel_spmd
from concourse.masks import make_identity

B, T, D = 8, 2048, 1024
P = 128
DC = D // P  # 8 feature chunks
TC = T // P  # 16 token chunks
NP = TC // 2  # 8 token-chunk pairs (DoubleRow)
NS = 512  # matmul moving free-dim
SL = T // NS  # 4 slices per full row
PW = 1024  # PSUM accumulator part width (2 banks)
F32 = mybir.dt.float32
BF16 = mybir.dt.bfloat16
FP8E4 = mybir.dt.float8e4  # e4m3
FP8E5 = mybir.dt.float8e5  # e5m2
NEG = -1.0e30
N_CORES = 8
WV_PRESCALE = 32.0  # keeps 32*Wv in e4m3's normal range (|Wv| ~ 0.02)
W_PRESCALE = 32.0  # same idea for 32*Wq, 32*Wk
DR = mybir.MatmulPerfMode.DoubleRow
COPY = mybir.ActivationFunctionType.Copy

NP_BF16 = mybir.dt.np(BF16)
NP_FP8E4 = mybir.dt.np(FP8E4)


def host_tri_mask() -> np.ndarray:
    """[128, 128] additive mask for the diagonal block of LT row-chunk jc:
    entry [p, c] (j = jc*128+p, i = jc*128+c) is 0 where i >= j else -1e30."""
    p = np.arange(P)[:, None]
    c = np.arange(P)[None, :]
    return np.where(c >= p, 0.0, NEG).astype(np.float32)


def build_nc(with_bias: bool):
    nc = bacc.Bacc("TRN2", target_bir_lowering=False, debug=False)

    x_d = nc.declare_dram_parameter("minibatch", [T, D], F32, isOutput=False)
    xb_d = nc.declare_dram_parameter("x_bf16", [T, D], BF16, isOutput=False)
    # out in bf16 (the host upcasts): halves the store stream, and the
    # residual dominates the output so the extra quantization is ~1e-3
    out_d = nc.declare_dram_parameter("out", [T, D], BF16, isOutput=True)
    if with_bias:
        tri_d = nc.declare_dram_parameter("tri_mask", [P, P], F32, isOutput=False)
        wq_d = nc.declare_dram_parameter("Wq", [D, D], F32, isOutput=False)
        bq_d = nc.declare_dram_parameter("bq", [D], F32, isOutput=False)
        wk_d = nc.declare_dram_parameter("Wk", [D, D], F32, isOutput=False)
        bk_d = nc.declare_dram_parameter("bk", [D], F32, isOutput=False)
        wv_d = nc.declare_dram_parameter("Wv", [D, D], F32, isOutput=False)
        bv_d = nc.declare_dram_parameter("bv", [D], F32, isOutput=False)
    else:
        xt8_d = nc.declare_dram_parameter("xt_fp8", [D, T], FP8E4, isOutput=False)
        wqt8_d = nc.declare_dram_parameter(
            "WqT32_fp8", [D, D], FP8E4, isOutput=False
        )
        wkt8_d = nc.declare_dram_parameter(
            "WkT32_fp8", [D, D], FP8E4, isOutput=False
        )
        wv8_d = nc.declare_dram_parameter("Wv_fp8", [D, D], FP8E4, isOutput=False)
        trib_d = nc.declare_dram_parameter(
            "tri_bf16", [P, P], BF16, isOutput=False
        )

    # no-bias: one 4-deep [P, PW] accumulator ring (8 banks) shared by the
    # C parts, E accs and deferred-V accs — reuse distance is then a full
    # V+C+E slot, past every softmax-chain consumer.  with_bias keeps a
    # split 2+1 layout (plus transpose bank pair).
    CB = 2 if with_bias else 4
    EB = 1  # with_bias-only E accumulator depth

    with tile.TileContext(nc) as tc:
        with (
            tc.tile_pool(name="persist", bufs=1) as persist,
            tc.tile_pool(name="wpool", bufs=8) as wpool,
            tc.tile_pool(name="fstage", bufs=4) as fstage,
            tc.tile_pool(name="stats", bufs=8) as stats,
            tc.tile_pool(name="psum_c", bufs=CB, space="PSUM") as cpool,
            tc.tile_pool(name="psum_e", bufs=EB, space="PSUM") as epool,
            tc.tile_pool(name="psum_t", bufs=2, space="PSUM") as tpool,
        ):
            def cacc():
                return cpool.tile([P, PW], F32, tag="acc", bufs=CB, name="acc")

            # ---- persistent activations ----
            # residual X rows, preloaded during the early (PE-bound) phases
            # so the C/E region's DMA bandwidth belongs to the output stores
            XRES = persist.tile([P, TC, D], BF16, tag="XRES", name="XRES")
            XT8 = persist.tile([P, DC, T], FP8E4, tag="XT8", name="XT8")
            V = persist.tile([P, TC, D], FP8E5, tag="V", name="V")  # V [j, v]
            Wv8 = persist.tile([P, DC, D], FP8E4, tag="Wv8", name="Wv8")
            # probs^T rows in pair-tiles for DoubleRow: pair p holds rows
            # jc=2p (at [:, 0, 0:]) and jc=2p+1 (at [:, 1, 128:]), both
            # covering i in [256*p, T).
            PT = [
                persist.tile(
                    [P, 2, T - 2 * P * p], FP8E5, tag=f"PT{p}", name=f"PT{p}"
                )
                for p in range(NP)
            ]
            # row 2p+1's first 128 columns are never written by exp but are
            # read by the pair matmuls -> must be zero.
            for p in range(NP):
                nc.gpsimd.memset(PT[p][:, 1, 0:P], 0.0)

            if with_bias:
                trimask = persist.tile([P, P], F32, tag="trimask", name="trimask")
                nc.sync.dma_start(out=trimask, in_=tri_d[:, :])
                for ic in range(TC):
                    nc.gpsimd.dma_start(
                        out=XRES[:, ic, :], in_=xb_d[ts(ic, P), :]
                    )
                XT = persist.tile([P, DC, T], BF16, tag="XT", name="XT")
                ones = persist.tile([1, NS], BF16, tag="ones", name="ones")
                nc.vector.memset(ones, 1.0)
                b_sb = {}
                for nm, bd in (("q", bq_d), ("k", bk_d), ("v", bv_d)):
                    bt = persist.tile(
                        [1, D], BF16, tag=f"bias_{nm}", name=f"bias_{nm}"
                    )
                    nc.gpsimd.dma_start(out=bt, in_=bd[None, :])  # f32 -> bf16
                    b_sb[nm] = bt
                nc.vector.tensor_scalar(
                    out=b_sb["v"],
                    in0=b_sb["v"],
                    scalar1=WV_PRESCALE,
                    scalar2=None,
                    op0=mybir.AluOpType.mult,
                )
                QT = persist.tile([P, DC, T], BF16, tag="QT", name="QT")
                KT = persist.tile([P, DC, T], BF16, tag="KT", name="KT")
                ident = persist.tile([P, P], BF16, tag="ident", name="ident")
                make_identity(nc, ident)

                for ic in range(TC):  # X^T via PE transpose
                    xf = fstage.tile(
                        [P, D], F32, tag="f32stage", bufs=4, name="xf"
                    )
                    nc.sync.dma_start(out=xf, in_=x_d[ts(ic, P), :])
                    xb = fstage.tile([P, D], BF16, tag="xbf", bufs=2, name="xb")
                    nc.vector.tensor_copy(out=xb, in_=xf)
                    pt_ = tpool.tile([P, D], BF16, tag="tacc", bufs=2, name="pt_")
                    for dc in range(DC):
                        nc.tensor.transpose(
                            pt_[:, ts(dc, P)], xb[:, ts(dc, P)], ident
                        )
                    nc.scalar.copy(
                        out=XT[:, :, ts(ic, P)],
                        in_=pt_.rearrange("p (dc c) -> p dc c", c=P),
                    )
                    nc.vector.tensor_copy(
                        out=XT8[:, :, ts(ic, P)], in_=XT[:, :, ts(ic, P)]
                    )

                def load_w_chunks(w_dram):
                    wt = []
                    for dc in range(DC):
                        w1 = wpool.tile([P, D], BF16, tag="w", bufs=8, name="w1")
                        nc.gpsimd.dma_start(out=w1, in_=w_dram[ts(dc, P), :])
                        wt.append(w1)
                    return wt

                for w_dram, bkey, dst in ((wq_d, "q", QT), (wk_d, "k", KT)):
                    wt = load_w_chunks(w_dram)
                    for m in range(DC):
                        for h in range(2):
                            acc = cacc()
                            for dc in range(DC):
                                for s in range(2 * h, 2 * h + 2):
                                    nc.tensor.matmul(
                                        acc[:, ts(s - 2 * h, NS)],
                                        lhsT=wt[dc][:, ts(m, P)],
                                        rhs=XT[:, dc, ts(s, NS)],
                                        start=(dc == 0),
                                        stop=False,
                                    )
                            for s in range(2):
                                nc.tensor.matmul(
                                    acc[:, ts(s, NS)],
                                    lhsT=b_sb[bkey][:, ts(m, P)],
                                    rhs=ones[:, :],
                                    start=False,
                                    stop=True,
                                )
                            nc.scalar.copy(
                                out=dst[:, m, ts(h, PW)], in_=acc
                            )

                wt = load_w_chunks(wv_d)
                for dc in range(DC):
                    nc.vector.tensor_scalar(
                        out=Wv8[:, dc, :],
                        in0=wt[dc][:, :],
                        scalar1=WV_PRESCALE,
                        scalar2=None,
                        op0=mybir.AluOpType.mult,
                    )

                # V (fp8 DoubleRow): acc[j, v] = 32 * sum_d X[j, d] Wv[d, v]
                for jc in range(TC):
                    acc = cacc()
                    for dp in range(DC // 2):
                        for vs in range(2):
                            nc.tensor.matmul(
                                acc[:, ts(vs, NS)],
                                lhsT=XT8[:, 2 * dp : 2 * dp + 2, ts(jc, P)],
                                rhs=Wv8[:, 2 * dp : 2 * dp + 2, ts(vs, NS)],
                                start=(dp == 0),
                                stop=False,
                                perf_mode=DR,
                            )
                    for vs in range(2):
                        nc.tensor.matmul(
                            acc[:, ts(vs, NS)],
                            lhsT=ones[:, 0:P],
                            rhs=b_sb["v"][:, ts(vs, NS)],
                            start=False,
                            stop=True,
                        )
                    nc.vector.tensor_copy(out=V[:, jc, :], in_=acc[:, 0:D])

                def c_step(acc_win, kidx, jc, lo, hi):
                    nc.tensor.matmul(
                        acc_win,
                        lhsT=KT[:, kidx, ts(jc, P)],
                        rhs=QT[:, kidx, lo:hi],
                        start=(kidx == 0),
                        stop=(kidx == DC - 1),
                    )

                n_csteps = DC

                def add_mask(acc, dstart):
                    nc.vector.tensor_add(
                        out=acc[:, dstart : dstart + P],
                        in0=acc[:, dstart : dstart + P],
                        in1=trimask,
                    )

            else:
                # everything arrives compute-ready (e4m3) from the host;
                # fp8 weights first so the M matmuls start immediately.
                WQT8 = persist.tile([P, DC, D], FP8E4, tag="WQT8", name="WQT8")
                WKT8 = persist.tile([P, DC, D], FP8E4, tag="WKT8", name="WKT8")
                identb = persist.tile([P, P], BF16, tag="identb", name="identb")
                make_identity(nc, identb)
                trib = persist.tile([P, P], BF16, tag="trib", name="trib")

                # Small PE warm-up (N=128 throwaways) so the activity monitor
                # is counting while the first weight pair streams in.
                warm = persist.tile([P, NS], BF16, tag="warm", name="warm")
                nc.vector.memset(warm, 0.0625)
                wacc = cacc()
                for _ in range(8):
                    nc.tensor.matmul(
                        wacc[:, 0:P],
                        lhsT=warm[:, 0:P],
                        rhs=warm[:, 0:P],
                        start=True,
                        stop=True,
                    )

                # M8 holds 32*M, M = Wq Wk^T:
                # acc[a, b] = sum_k 32Wq[a, k] * 32Wk[b, k] = 1024 M[a, b]
                # First half (a=0..3) holds all four accumulators live in the
                # PSUM ring and interleaves each contraction pair's matmuls
                # with the NEXT pair's DMA enqueue: a consumer's DMA wait
                # counts every earlier descriptor on its queue, so this trace
                # order lets pair-k matmuls run as soon as pair k lands
                # instead of waiting out the whole 2MB weight stream.
                M8 = persist.tile([P, DC, D], FP8E4, tag="M8", name="M8")
                NA = 4  # accumulators live at once = cpool depth
                maccs = [cacc() for _ in range(NA)]
                for kp in range(DC // 2):
                    for r in range(2):
                        nc.sync.dma_start(
                            out=WQT8[:, 2 * kp + r, :],
                            in_=wqt8_d[ts(2 * kp + r, P), :],
                        )
                        nc.gpsimd.dma_start(
                            out=WKT8[:, 2 * kp + r, :],
                            in_=wkt8_d[ts(2 * kp + r, P), :],
                        )
                    for a in range(NA):
                        for bs in range(2):
                            nc.tensor.matmul(
                                maccs[a][:, ts(bs, NS)],
                                lhsT=WQT8[:, 2 * kp : 2 * kp + 2, ts(a, P)],
                                rhs=WKT8[:, 2 * kp : 2 * kp + 2, ts(bs, NS)],
                                start=(kp == 0),
                                stop=(kp == DC // 2 - 1),
                                perf_mode=DR,
                            )
                for a in range(NA):
                    nc.scalar.activation(
                        out=M8[:, a, :],
                        in_=maccs[a],
                        func=COPY,
                        scale=1.0 / 32.0,
                    )
                nc.sync.dma_start(out=trib, in_=trib_d[:, :])
                for a in range(NA, DC):
                    acc = cacc()
                    for kp in range(DC // 2):
                        for bs in range(2):
                            nc.tensor.matmul(
                                acc[:, ts(bs, NS)],
                                lhsT=WQT8[:, 2 * kp : 2 * kp + 2, ts(a, P)],
                                rhs=WKT8[:, 2 * kp : 2 * kp + 2, ts(bs, NS)],
                                start=(kp == 0),
                                stop=(kp == DC // 2 - 1),
                                perf_mode=DR,
                            )
                    nc.scalar.activation(
                        out=M8[:, a, :], in_=acc, func=COPY, scale=1.0 / 32.0
                    )

                # X^T / Wv / residual-X loads, batched into 512KB descriptors
                # (fewer completion semaphores -> shorter end-of-kernel
                # drain) and traced AFTER the M matmuls: a consumer's DMA
                # wait counts every earlier descriptor on its queue, so the
                # slow batched transfers must not sit ahead of M's weights.
                for g in range(4):
                    nc.gpsimd.dma_start(
                        out=XT8[:, 2 * g : 2 * g + 2, :],
                        in_=xt8_d[256 * g : 256 * (g + 1), :].rearrange(
                            "(c p) t -> p c t", p=P
                        ),
                    )
                for g in range(2):
                    nc.sync.dma_start(
                        out=Wv8[:, 4 * g : 4 * g + 4, :],
                        in_=wv8_d[512 * g : 512 * (g + 1), :].rearrange(
                            "(c p) d -> p c d", p=P
                        ),
                    )
                for g in range(4):
                    nc.gpsimd.dma_start(
                        out=XRES[:, 4 * g : 4 * g + 4, :],
                        in_=xb_d[512 * g : 512 * (g + 1), :].rearrange(
                            "(c p) d -> p c d", p=P
                        ),
                    )

                # V (fp8 DoubleRow): acc[j, v] = 32 * sum_d X[j, d] Wv[d, v];
                # cast to e5m2 on the Scalar engine (Vector stays free for
                # the softmax chain).  Rows 0..5 up front; rows 6..15 are
                # deferred into the C/E region as PE filler (they have no
                # softmax dependency, so they absorb the PSUM-ring waits).
                def phase_v(jc):
                    acc = cacc()
                    for dp in range(DC // 2):
                        for vs in range(2):
                            nc.tensor.matmul(
                                acc[:, ts(vs, NS)],
                                lhsT=XT8[:, 2 * dp : 2 * dp + 2, ts(jc, P)],
                                rhs=Wv8[:, 2 * dp : 2 * dp + 2, ts(vs, NS)],
                                start=(dp == 0),
                                stop=(dp == DC // 2 - 1),
                                perf_mode=DR,
                            )
                    nc.scalar.copy(out=V[:, jc, :], in_=acc)

                n_v_early = 2
                for jc in range(n_v_early):
                    phase_v(jc)

                # Y^T (stored as Y, e4m3): acc[e, i] = sum_d 32M[d, e] X[i, d]
                YT8 = persist.tile([P, DC, T], FP8E4, tag="YT8", name="YT8")
                for m in range(DC):
                    for h in range(2):
                        acc = cacc()
                        for dp in range(DC // 2):
                            for s in range(2 * h, 2 * h + 2):
                                nc.tensor.matmul(
                                    acc[:, ts(s - 2 * h, NS)],
                                    lhsT=M8[:, 2 * dp : 2 * dp + 2, ts(m, P)],
                                    rhs=XT8[:, 2 * dp : 2 * dp + 2, ts(s, NS)],
                                    start=(dp == 0),
                                    stop=(dp == DC // 2 - 1),
                                    perf_mode=DR,
                                )
                        nc.scalar.activation(
                            out=YT8[:, m, ts(h, PW)],
                            in_=acc,
                            func=COPY,
                            scale=1.0 / 32.0,
                        )

                def c_step(acc_win, kidx, jc, lo, hi):
                    # LT[j, i] = sum_e X[j, e] Y[i, e], fp8 DoubleRow
                    nc.tensor.matmul(
                        acc_win,
                        lhsT=XT8[:, 2 * kidx : 2 * kidx + 2, ts(jc, P)],
                        rhs=YT8[:, 2 * kidx : 2 * kidx + 2, lo:hi],
                        start=(kidx == 0),
                        stop=(kidx == DC // 2 - 1),
                        perf_mode=DR,
                    )

                n_csteps = DC // 2

                def add_mask(acc, dstart):
                    # acc[:, dstart:dstart+P] += I^T @ trib = trib, on the PE
                    nc.tensor.matmul(
                        acc[:, dstart : dstart + P],
                        lhsT=identb,
                        rhs=trib,
                        start=False,
                        stop=True,
                        skip_group_check=True,
                    )

            # ====== phases C+D+E interleaved ======
            # C_jc: LT row-chunk jc (i >= 128*jc) in 1-2 PSUM parts + softmax
            # E_ic: read[ic] = probs @ V + residual + store
            def phase_c(jc):
                g, r = jc // 4, jc % 4
                dstart = P * r  # diagonal block offset within slice g
                pr, rr = jc // 2, jc % 2
                # parts: slice ranges [g, g+2) and [g+2, 4)
                parts = [(g, min(g + 2, SL))]
                if g + 2 < SL:
                    parts.append((g + 2, SL))
                nm_parts = []
                accs = []
                for sl_lo, sl_hi in parts:
                    acc = cacc()
                    accs.append(acc)
                    for kidx in range(n_csteps):
                        for s in range(sl_lo, sl_hi):
                            lo = NS * s if s > g else P * jc
                            hi = NS * (s + 1)
                            c_step(
                                acc[:, lo - NS * sl_lo : hi - NS * sl_lo],
                                kidx,
                                jc,
                                lo,
                                hi,
                            )
                    if sl_lo == g:
                        add_mask(acc, dstart)
                    nm = stats.tile([P, 1], F32, tag="nm", bufs=8, name="nm")
                    lo_l = dstart if sl_lo == g else 0
                    nc.vector.reduce_max(
                        out=nm,
                        in_=acc[:, lo_l : NS * (sl_hi - sl_lo)],
                        axis=mybir.AxisListType.X,
                        negate=True,
                    )
                    nm_parts.append(nm)
                if len(nm_parts) == 2:
                    negmax = stats.tile(
                        [P, 1], F32, tag="nm", bufs=8, name="nmc"
                    )
                    nc.vector.tensor_tensor(
                        out=negmax,
                        in0=nm_parts[0],
                        in1=nm_parts[1],
                        op=mybir.AluOpType.min,
                    )
                else:
                    negmax = nm_parts[0]
                ss_parts = []
                for (sl_lo, sl_hi), acc in zip(parts, accs):
                    ssum = stats.tile(
                        [P, 1], F32, tag="ssum", bufs=8, name="ssum"
                    )
                    lo_l = dstart if sl_lo == g else 0
                    base = P * rr + (NS * sl_lo + lo_l - P * jc)
                    width = NS * (sl_hi - sl_lo) - lo_l
                    nc.scalar.activation(
                        out=PT[pr][:, rr, base : base + width],
                        in_=acc[:, lo_l : NS * (sl_hi - sl_lo)],
                        func=mybir.ActivationFunctionType.Exp,
                        bias=negmax,
                        scale=1.0,
                        accum_out=ssum,
                    )
                    ss_parts.append(ssum)
                if len(ss_parts) == 2:
                    stot = stats.tile(
                        [P, 1], F32, tag="ssum", bufs=8, name="stot"
                    )
                    nc.vector.tensor_add(
                        out=stot, in0=ss_parts[0], in1=ss_parts[1]
                    )
                else:
                    stot = ss_parts[0]
                rv = stats.tile([P, 1], F32, tag="rv", bufs=4, name="rv")
                nc.vector.reciprocal(out=rv, in_=stot)
                # fold softmax denominator, 1/sqrt(1024) and the 1/32 weight
                # pre-scale compensation into V's rows: V[j, :] *= rv[j]/1024
                nc.vector.tensor_scalar(
                    out=V[:, jc, :],
                    in0=V[:, jc, :],
                    scalar1=rv,
                    scalar2=1.0 / (32.0 * WV_PRESCALE),
                    op0=mybir.AluOpType.mult,
                    op1=mybir.AluOpType.mult,
                )

            def phase_e(ic):
                if with_bias:
                    acc = epool.tile(
                        [P, PW], F32, tag="eacc", bufs=EB, name="eacc"
                    )
                else:
                    acc = cacc()
                np_ic = ic // 2 + 1  # pairs 0..ic//2
                for p in range(np_ic):
                    blk = PT[p][
                        :, :, ic * P - 2 * P * p : (ic + 1) * P - 2 * P * p
                    ]
                    for vs in range(2):
                        nc.tensor.matmul(
                            acc[:, ts(vs, NS)],
                            lhsT=blk,
                            rhs=V[:, 2 * p : 2 * p + 2, ts(vs, NS)],
                            start=(p == 0),
                            stop=(p == np_ic - 1),
                            perf_mode=DR,
                        )
                ot = fstage.tile([P, D], BF16, tag="xres", bufs=4, name="ot")
                if ic >= TC - 2:
                    # kernel tail: pipeline the residual add with the store
                    for h in range(2):
                        nc.vector.tensor_add(
                            out=ot[:, ts(h, NS)],
                            in0=acc[:, ts(h, NS)],
                            in1=XRES[:, ic, ts(h, NS)],
                        )
                        nc.sync.dma_start(
                            out=out_d[ts(ic, P), ts(h, NS)],
                            in_=ot[:, ts(h, NS)],
                        )
                else:
                    nc.vector.tensor_add(out=ot, in0=acc, in1=XRES[:, ic, :])
                    nc.sync.dma_start(out=out_d[ts(ic, P), :], in_=ot)

            # lag-3 interleave: E_k needs V row k+1 scaled (end of C_{k+1}'s
            # softmax chain); tracing E_{t-3} after C_t leaves two C phases
            # of PE work between a softmax chain and the E that needs it.
            # Deferred V rows slot in ahead of each C row as ring-wait
            # filler; the post-loop E phases borrow the (by then idle) C
            # pool so they don't serialize on a single E accumulator.
            LAG = 3
            v_next = n_v_early if not with_bias else TC
            for jc in range(TC):
                if jc >= 1 and v_next < TC:
                    phase_v(v_next)
                    v_next += 1
                phase_c(jc)
                if jc >= LAG:
                    phase_e(jc - LAG)
            for ic in range(TC - LAG, TC):
                phase_e(ic)

    nc.finalize()
    return nc


_NC_CACHE = {}


def get_nc(with_bias: bool = False):
    if with_bias not in _NC_CACHE:
        _NC_CACHE[with_bias] = build_nc(with_bias)
    return _NC_CACHE[with_bias]


def make_in_maps(inputs: dict) -> list[dict]:
    mb = np.ascontiguousarray(np.asarray(inputs["minibatch"], dtype=np.float32))
    assert mb.shape == (B, T, D)
    shared = {
        k: np.ascontiguousarray(np.asarray(inputs[k], dtype=np.float32))
        for k in ("Wq", "bq", "Wk", "bk", "Wv", "bv")
    }
    shared["tri_mask"] = host_tri_mask()
    shared["tri_bf16"] = host_tri_mask().astype(NP_BF16)
    # alternate layouts/dtypes of the same inputs -> no device transposes
    # or dtype-conversion passes
    shared["WqT32_fp8"] = np.ascontiguousarray(
        (W_PRESCALE * shared["Wq"]).T
    ).astype(NP_FP8E4)
    shared["WkT32_fp8"] = np.ascontiguousarray(
        (W_PRESCALE * shared["Wk"]).T
    ).astype(NP_FP8E4)
    shared["Wv_fp8"] = (shared["Wv"] * WV_PRESCALE).astype(NP_FP8E4)
    maps = []
    for c in range(N_CORES):
        xt = np.ascontiguousarray(mb[c].T)
        maps.append(
            {
                "minibatch": mb[c],
                "x_bf16": mb[c].astype(NP_BF16),
                "xt_fp8": xt.astype(NP_FP8E4),
                **shared,
            }
        )
    return maps


def needs_bias(inputs: dict) -> bool:
    return any(
        np.any(np.asarray(inputs[k], dtype=np.float32) != 0.0)
        for k in ("bq", "bk", "bv")
    )


def kernel(**inputs) -> np.ndarray:
    nc = get_nc(with_bias=needs_bias(inputs))
    in_maps = make_in_maps(inputs)
    res = run_bass_kernel_spmd(nc, in_maps, core_ids=list(range(N_CORES)))
    return np.stack(
        [res.results[c]["out"].astype(np.float32) for c in range(N_CORES)],
        axis=0,
    )


if __name__ == "__main__":
    rng = np.random.default_rng(0)
    demo = {
        "minibatch": rng.standard_normal((B, T, D), dtype=np.float32),
        "Wq": rng.standard_normal((D, D), dtype=np.float32) * 0.02,
        "bq": np.zeros(D, np.float32),
        "Wk": rng.standard_normal((D, D), dtype=np.float32) * 0.02,
        "bk": np.zeros(D, np.float32),
        "Wv": rng.standard_normal((D, D), dtype=np.float32) * 0.02,
        "bv": np.zeros(D, np.float32),
    }
    out = kernel(**demo)
    print(out.shape, out.dtype)
